# revision 5
# baseline (speedup 1.0000x reference)
"""MoE (top-2 of 8 experts, SwiGLU) Trainium2 kernel — expert-parallel over 8 NeuronCores.

Strategy
--------
- Host computes the tiny router (T x D @ D x 8 = 0.03% of total FLOPs) with the
  exact same jax ops as the reference, so top-k selection matches bitwise.
- Token dispatch ("all-to-all") happens on the host: tokens routed to expert e
  are gathered (transposed, capacity-padded) and shipped to core e.
- Each of the 8 cores runs an identical SPMD Bass program: the SwiGLU expert
  FFN for its expert over its capacity-C token slab.  Matmuls use float32r
  (full fp32 data; streams at 1 cycle/row for moving dim >= 256, i.e. bf16
  speed with fp32-grade precision).
- Host scatter-adds the two expert outputs per token back together with the
  routing weights (exactly the reference's dense-combine math restricted to
  the nonzero entries).

Per-core device work: ~C x (3 matmuls of [*,1024]x[1024,1024]) ≈ 2.6e10 FLOPs
(sparse: 4x less than the reference's dense form), vs 1.03e11 dense.
"""

import math
from contextlib import ExitStack

import numpy as np

import concourse.bass as bass
import concourse.tile as tile
from concourse import bacc, mybir
from concourse.bass_utils import run_bass_kernel_spmd

D = 1024       # model dim
H = 1024       # per-expert hidden dim
E = 8          # experts == cores
TG = 512       # tokens per tile chunk (full PSUM bank; float32r full rate >= 256)
MM_DT = mybir.dt.float32r
ACC_DT = mybir.dt.float32

_BUILD_CACHE: dict[int, object] = {}


def _chunks(C):
    """Split C tokens into chunks of 512 plus one ragged 128-multiple tail."""
    out = [TG] * (C // TG)
    if C % TG:
        out.append(C % TG)
    return out


def _build(C):
    """Build + compile the single-expert SwiGLU FFN program for capacity C.

    Computes yt = (silu(xt.T @ pw) * (xt.T @ gw)) @ ow, transposed:
    everything is laid out [feature, token] so no on-device transposes are
    needed (host ships x pre-transposed and un-transposes y).
    """
    assert C % 128 == 0
    nc = bacc.Bacc("TRN2", target_bir_lowering=False, debug=False, num_devices=E)
    xt = nc.dram_tensor("xt", [D, C], MM_DT, kind="ExternalInput").ap()
    gw = nc.dram_tensor("gw", [D, H], MM_DT, kind="ExternalInput").ap()
    pw = nc.dram_tensor("pw", [D, H], MM_DT, kind="ExternalInput").ap()
    ow = nc.dram_tensor("ow", [H, D], MM_DT, kind="ExternalInput").ap()
    yt = nc.dram_tensor("yt", [D, C], ACC_DT, kind="ExternalOutput").ap()

    KB = D // 128  # contraction blocks for the first matmuls
    HB = H // 128  # hidden blocks

    with tile.TileContext(nc) as tc, ExitStack() as ctx:
        wpool = ctx.enter_context(tc.tile_pool(name="w", bufs=1))
        xpool = ctx.enter_context(tc.tile_pool(name="x", bufs=2))
        hpool = ctx.enter_context(tc.tile_pool(name="h", bufs=2))
        spool = ctx.enter_context(tc.tile_pool(name="s", bufs=2))
        ypool = ctx.enter_context(tc.tile_pool(name="y", bufs=3))
        gpsum = ctx.enter_context(tc.tile_pool(name="pg", bufs=2, space="PSUM"))
        ppsum = ctx.enter_context(tc.tile_pool(name="pp", bufs=2, space="PSUM"))
        ypsum = ctx.enter_context(tc.tile_pool(name="py", bufs=2, space="PSUM"))

        def load_x(col, tg):
            xs = []
            for k in range(KB):
                t = xpool.tile([128, tg], MM_DT, tag=f"x{k}")
                nc.sync.dma_start(t[:], xt[k * 128:(k + 1) * 128, col:col + tg])
                xs.append(t)
            return xs

        def load_w(ap_, prefix):
            ts = []
            for k in range(KB):
                t = wpool.tile([128, H], MM_DT, tag=f"{prefix}{k}")
                nc.sync.dma_start(t[:], ap_[k * 128:(k + 1) * 128, :])
                ts.append(t)
            return ts

        # Issue order matters: PE can start on chunk 0's G matmuls as soon as
        # x(chunk0) + gw arrive; pw is needed ~14us later, ow ~27us later.
        chunk_list = _chunks(C)
        xs_next = load_x(0, chunk_list[0])
        gw_t = load_w(gw, "gw")
        pw_t = load_w(pw, "pw")
        ow_t = load_w(ow, "ow")

        col = 0
        for ci, tg in enumerate(chunk_list):
            xs = xs_next
            if ci + 1 < len(chunk_list):
                xs_next = load_x(col + tg, chunk_list[ci + 1])

            hs = []
            for h in range(HB):
                pg = gpsum.tile([128, tg], ACC_DT, tag="pg")
                for k in range(KB):
                    nc.tensor.matmul(
                        pg[:], gw_t[k][:, h * 128:(h + 1) * 128], xs[k][:],
                        start=(k == 0), stop=(k == KB - 1))
                pp = ppsum.tile([128, tg], ACC_DT, tag="pp")
                for k in range(KB):
                    nc.tensor.matmul(
                        pp[:], pw_t[k][:, h * 128:(h + 1) * 128], xs[k][:],
                        start=(k == 0), stop=(k == KB - 1))
                sg = spool.tile([128, tg], ACC_DT, tag="sig")
                nc.scalar.activation(sg[:], pp[:], mybir.ActivationFunctionType.Sigmoid)
                sl = spool.tile([128, tg], ACC_DT, tag="sil")
                nc.vector.tensor_mul(sl[:], pp[:], sg[:])   # silu(p), one PSUM read
                ht = hpool.tile([128, tg], MM_DT, tag=f"h{h}")
                nc.vector.tensor_mul(ht[:], pg[:], sl[:])
                hs.append(ht)

            for dblk in range(KB):
                py = ypsum.tile([128, tg], ACC_DT, tag="py")
                for h in range(HB):
                    nc.tensor.matmul(
                        py[:], ow_t[h][:, dblk * 128:(dblk + 1) * 128], hs[h][:],
                        start=(h == 0), stop=(h == HB - 1))
                ysb = ypool.tile([128, tg], ACC_DT, tag="y")
                nc.vector.tensor_copy(ysb[:], py[:])
                # gpsimd queue: keep SyncE's HWDGE queue free for input feeding
                nc.gpsimd.dma_start(yt[dblk * 128:(dblk + 1) * 128, col:col + tg], ysb[:])
            col += tg

    nc.compile()
    return nc


def _get_program(C):
    if C not in _BUILD_CACHE:
        _BUILD_CACHE[C] = _build(C)
    return _BUILD_CACHE[C]


def _route(x, gate_w, k):
    """Router with the reference's exact jax ops (bitwise-matching top-k)."""
    import jax
    import jax.numpy as jnp

    router_logits = jnp.asarray(x) @ jnp.asarray(gate_w)
    routing_probs = jax.nn.softmax(router_logits.astype(jnp.float32), axis=-1)
    top_w, top_i = jax.lax.top_k(routing_probs, k)
    top_w = top_w / jnp.sum(top_w, axis=-1, keepdims=True)
    top_w = top_w.astype(jnp.float32)

    n_exp = gate_w.shape[1]
    expert_mask = jax.nn.one_hot(top_i, n_exp, dtype=jnp.float32)
    tokens_per_expert = jnp.mean(expert_mask, axis=0)
    router_prob_per_expert = jnp.mean(routing_probs, axis=0)
    bl_loss = jnp.sum(tokens_per_expert * router_prob_per_expert[None, :]) * n_exp

    return (np.asarray(router_logits), np.asarray(top_i), np.asarray(top_w),
            np.asarray(bl_loss))


def _run_device(nc, in_maps, trace=False, **kw):
    return run_bass_kernel_spmd(nc, in_maps, core_ids=list(range(E)),
                                trace=trace, **kw)


def kernel(hidden_states, gate_w, gw, pw, ow, top_k, _trace=False, _res_out=None):
    hidden_states = np.asarray(hidden_states, dtype=np.float32)
    gate_w = np.asarray(gate_w, dtype=np.float32)
    gw = np.ascontiguousarray(np.asarray(gw, dtype=np.float32))
    pw = np.ascontiguousarray(np.asarray(pw, dtype=np.float32))
    ow = np.ascontiguousarray(np.asarray(ow, dtype=np.float32))
    k = int(top_k)

    B, S, _ = hidden_states.shape
    x = hidden_states.reshape(-1, D)
    T = x.shape[0]

    router_logits, top_i, top_w, bl_loss = _route(x, gate_w, k)

    # --- host dispatch (the "all-to-all") ---
    counts = np.bincount(top_i.ravel(), minlength=E)
    C = max(128, int(math.ceil(counts.max() / 128)) * 128)
    prog = _get_program(C)

    xT = np.ascontiguousarray(x.T)  # [D, T] so per-expert gathers are column slices
    idx_list, w_list, in_maps = [], [], []
    for e in range(E):
        sel = top_i == e                       # [T, k]
        idx = np.nonzero(sel.any(axis=1))[0]   # tokens routed to e
        wgt = (top_w[idx] * sel[idx]).sum(axis=1).astype(np.float32)
        xt = np.zeros((D, C), np.float32)
        xt[:, :idx.size] = xT[:, idx]
        idx_list.append(idx)
        w_list.append(wgt)
        in_maps.append({"xt": xt, "gw": gw[e], "pw": pw[e], "ow": ow[e]})

    res = _run_device(prog, in_maps, trace=_trace)
    if _res_out is not None:
        _res_out.append(res)

    final = np.zeros((T, D), np.float32)
    for e in range(E):
        idx = idx_list[e]
        ye = res.results[e]["yt"][:, :idx.size].T  # [n_e, D]
        final[idx] += ye * w_list[e][:, None]

    return (final.reshape(B, S, D),
            router_logits,
            np.float32(bl_loss))


# revision 6
# speedup vs baseline: 1.0638x; 1.0638x over previous
"""MoE (top-2 of 8 experts, SwiGLU) Trainium2 kernel — expert-parallel over 8 NeuronCores.

Strategy
--------
- Host computes the tiny router (T x D @ D x 8 = 0.03% of total FLOPs) with the
  exact same jax ops as the reference, so top-k selection matches bitwise.
- Token dispatch ("all-to-all") happens on the host: tokens routed to expert e
  are gathered (transposed, capacity-padded) and shipped to core e.
- Each of the 8 cores runs an identical SPMD Bass program: the SwiGLU expert
  FFN for its expert over its capacity-C token slab.  Matmuls use float32r
  (full fp32 data; streams at 1 cycle/row for moving dim >= 256, i.e. bf16
  speed with fp32-grade precision).
- Host scatter-adds the two expert outputs per token back together with the
  routing weights (exactly the reference's dense-combine math restricted to
  the nonzero entries).

Per-core device work: ~C x (3 matmuls of [*,1024]x[1024,1024]) ≈ 2.6e10 FLOPs
(sparse: 4x less than the reference's dense form), vs 1.03e11 dense.
"""

import math
from contextlib import ExitStack

import numpy as np

import concourse.bass as bass
import concourse.tile as tile
from concourse import bacc, mybir
from concourse.bass_utils import run_bass_kernel_spmd

D = 1024       # model dim
H = 1024       # per-expert hidden dim
E = 8          # experts == cores
TG = 512       # tokens per tile chunk (full PSUM bank; float32r full rate >= 256)
MM_DT = mybir.dt.float32r
ACC_DT = mybir.dt.float32

_BUILD_CACHE: dict[int, object] = {}


def _chunks(C):
    """Split C tokens into chunks of 512 plus one ragged 128-multiple tail."""
    out = [TG] * (C // TG)
    if C % TG:
        out.append(C % TG)
    return out


def _build(C):
    """Build + compile the single-expert SwiGLU FFN program for capacity C.

    Computes yt = (silu(xt.T @ pw) * (xt.T @ gw)) @ ow, transposed:
    everything is laid out [feature, token] so no on-device transposes are
    needed (host ships x pre-transposed and un-transposes y).
    """
    assert C % 128 == 0
    nc = bacc.Bacc("TRN2", target_bir_lowering=False, debug=False, num_devices=E)
    xt = nc.dram_tensor("xt", [D, C], MM_DT, kind="ExternalInput").ap()
    gw = nc.dram_tensor("gw", [D, H], MM_DT, kind="ExternalInput").ap()
    pw = nc.dram_tensor("pw", [D, H], MM_DT, kind="ExternalInput").ap()
    ow = nc.dram_tensor("ow", [H, D], MM_DT, kind="ExternalInput").ap()
    yt = nc.dram_tensor("yt", [D, C], ACC_DT, kind="ExternalOutput").ap()

    KB = D // 128  # contraction blocks for the first matmuls
    HB = H // 128  # hidden blocks

    # Partition-blocked 3D views: [(a p) m] -> [p, a, m] so each weight
    # matrix / token chunk moves as ONE large DMA (HWDGE trigger is ~600ns
    # of engine time each; batching to >=1MiB is the documented rule).
    xt_r = xt.rearrange("(a p) c -> p a c", p=128)
    yt_r = yt.rearrange("(a p) c -> p a c", p=128)
    gw_r = gw.rearrange("(a p) m -> p a m", p=128)
    pw_r = pw.rearrange("(a p) m -> p a m", p=128)
    ow_r = ow.rearrange("(a p) m -> p a m", p=128)

    with tile.TileContext(nc) as tc, ExitStack() as ctx:
        wpool = ctx.enter_context(tc.tile_pool(name="w", bufs=1))
        xpool = ctx.enter_context(tc.tile_pool(name="x", bufs=2))
        hpool = ctx.enter_context(tc.tile_pool(name="h", bufs=2))
        spool = ctx.enter_context(tc.tile_pool(name="s", bufs=2))
        ypool = ctx.enter_context(tc.tile_pool(name="y", bufs=2))
        gpsum = ctx.enter_context(tc.tile_pool(name="pg", bufs=2, space="PSUM"))
        ppsum = ctx.enter_context(tc.tile_pool(name="pp", bufs=2, space="PSUM"))
        ypsum = ctx.enter_context(tc.tile_pool(name="py", bufs=2, space="PSUM"))

        def load_x(col, tg):
            t = xpool.tile([128, KB, tg], MM_DT, tag="x")
            nc.sync.dma_start(t[:], xt_r[:, :, col:col + tg])
            return t

        def load_w(ap_r, prefix):
            t = wpool.tile([128, KB, H], MM_DT, tag=prefix)
            nc.sync.dma_start(t[:], ap_r[:, :, :])
            return t

        # Issue order matters: PE starts on chunk 0's G matmuls as soon as
        # x(chunk0) + gw arrive; pw is needed ~14us later, ow ~28us later
        # (and the Y phase runs one chunk behind to hide ow's wire time).
        chunk_list = _chunks(C)
        xs_next = load_x(0, chunk_list[0])
        gw_t = load_w(gw_r, "gw")
        pw_t = load_w(pw_r, "pw")
        ow_t = load_w(ow_r, "ow")

        pending_y = None  # (hs, col, tg) of the previous chunk

        def emit_y(hs, ycol, tg):
            ybig = ypool.tile([128, KB, tg], ACC_DT, tag="y")
            for dblk in range(KB):
                py = ypsum.tile([128, tg], ACC_DT, tag="py")
                for h in range(HB):
                    nc.tensor.matmul(
                        py[:], ow_t[:, h, dblk * 128:(dblk + 1) * 128], hs[h][:],
                        start=(h == 0), stop=(h == HB - 1))
                nc.vector.tensor_copy(ybig[:, dblk, :], py[:])
            # ACT's HWDGE ring: keep SyncE's ring free for input feeding
            nc.scalar.dma_start(yt_r[:, :, ycol:ycol + tg], ybig[:])

        col = 0
        for ci, tg in enumerate(chunk_list):
            xs = xs_next
            if ci + 1 < len(chunk_list):
                xs_next = load_x(col + tg, chunk_list[ci + 1])

            hs = []
            for h in range(HB):
                pg = gpsum.tile([128, tg], ACC_DT, tag="pg")
                for k in range(KB):
                    nc.tensor.matmul(
                        pg[:], gw_t[:, k, h * 128:(h + 1) * 128], xs[:, k, :],
                        start=(k == 0), stop=(k == KB - 1))
                pp = ppsum.tile([128, tg], ACC_DT, tag="pp")
                for k in range(KB):
                    nc.tensor.matmul(
                        pp[:], pw_t[:, k, h * 128:(h + 1) * 128], xs[:, k, :],
                        start=(k == 0), stop=(k == KB - 1))
                sg = spool.tile([128, tg], ACC_DT, tag="sig")
                nc.scalar.activation(sg[:], pp[:], mybir.ActivationFunctionType.Sigmoid)
                sl = spool.tile([128, tg], ACC_DT, tag="sil")
                nc.vector.tensor_mul(sl[:], pp[:], sg[:])   # silu(p), one PSUM read
                ht = hpool.tile([128, tg], MM_DT, tag=f"h{h}")
                nc.vector.tensor_mul(ht[:], pg[:], sl[:])
                hs.append(ht)

            if pending_y is not None:
                emit_y(*pending_y)
            pending_y = (hs, col, tg)
            col += tg
        emit_y(*pending_y)

    nc.compile()
    return nc


def _get_program(C):
    if C not in _BUILD_CACHE:
        _BUILD_CACHE[C] = _build(C)
    return _BUILD_CACHE[C]


def _route(x, gate_w, k):
    """Router with the reference's exact jax ops (bitwise-matching top-k)."""
    import jax
    import jax.numpy as jnp

    router_logits = jnp.asarray(x) @ jnp.asarray(gate_w)
    routing_probs = jax.nn.softmax(router_logits.astype(jnp.float32), axis=-1)
    top_w, top_i = jax.lax.top_k(routing_probs, k)
    top_w = top_w / jnp.sum(top_w, axis=-1, keepdims=True)
    top_w = top_w.astype(jnp.float32)

    n_exp = gate_w.shape[1]
    expert_mask = jax.nn.one_hot(top_i, n_exp, dtype=jnp.float32)
    tokens_per_expert = jnp.mean(expert_mask, axis=0)
    router_prob_per_expert = jnp.mean(routing_probs, axis=0)
    bl_loss = jnp.sum(tokens_per_expert * router_prob_per_expert[None, :]) * n_exp

    return (np.asarray(router_logits), np.asarray(top_i), np.asarray(top_w),
            np.asarray(bl_loss))


def _run_device(nc, in_maps, trace=False, **kw):
    return run_bass_kernel_spmd(nc, in_maps, core_ids=list(range(E)),
                                trace=trace, **kw)


def kernel(hidden_states, gate_w, gw, pw, ow, top_k, _trace=False, _res_out=None):
    hidden_states = np.asarray(hidden_states, dtype=np.float32)
    gate_w = np.asarray(gate_w, dtype=np.float32)
    gw = np.ascontiguousarray(np.asarray(gw, dtype=np.float32))
    pw = np.ascontiguousarray(np.asarray(pw, dtype=np.float32))
    ow = np.ascontiguousarray(np.asarray(ow, dtype=np.float32))
    k = int(top_k)

    B, S, _ = hidden_states.shape
    x = hidden_states.reshape(-1, D)
    T = x.shape[0]

    router_logits, top_i, top_w, bl_loss = _route(x, gate_w, k)

    # --- host dispatch (the "all-to-all") ---
    counts = np.bincount(top_i.ravel(), minlength=E)
    C = max(128, int(math.ceil(counts.max() / 128)) * 128)
    prog = _get_program(C)

    xT = np.ascontiguousarray(x.T)  # [D, T] so per-expert gathers are column slices
    idx_list, w_list, in_maps = [], [], []
    for e in range(E):
        sel = top_i == e                       # [T, k]
        idx = np.nonzero(sel.any(axis=1))[0]   # tokens routed to e
        wgt = (top_w[idx] * sel[idx]).sum(axis=1).astype(np.float32)
        xt = np.zeros((D, C), np.float32)
        xt[:, :idx.size] = xT[:, idx]
        idx_list.append(idx)
        w_list.append(wgt)
        in_maps.append({"xt": xt, "gw": gw[e], "pw": pw[e], "ow": ow[e]})

    res = _run_device(prog, in_maps, trace=_trace)
    if _res_out is not None:
        _res_out.append(res)

    final = np.zeros((T, D), np.float32)
    for e in range(E):
        idx = idx_list[e]
        ye = res.results[e]["yt"][:, :idx.size].T  # [n_e, D]
        final[idx] += ye * w_list[e][:, None]

    return (final.reshape(B, S, D),
            router_logits,
            np.float32(bl_loss))


# revision 8
# speedup vs baseline: 1.0807x; 1.0160x over previous
"""MoE (top-2 of 8 experts, SwiGLU) Trainium2 kernel — expert-parallel over 8 NeuronCores.

Strategy
--------
- Host computes the tiny router (T x D @ D x 8 = 0.03% of total FLOPs) with the
  exact same jax ops as the reference, so top-k selection matches bitwise.
- Token dispatch ("all-to-all") happens on the host: tokens routed to expert e
  are gathered (transposed, capacity-padded) and shipped to core e.
- Each of the 8 cores runs an identical SPMD Bass program: the SwiGLU expert
  FFN for its expert over its capacity-C token slab.  Matmuls use float32r
  (full fp32 data; streams at 1 cycle/row for moving dim >= 256, i.e. bf16
  speed with fp32-grade precision).
- Host scatter-adds the two expert outputs per token back together with the
  routing weights (exactly the reference's dense-combine math restricted to
  the nonzero entries).

Per-core device work: ~C x (3 matmuls of [*,1024]x[1024,1024]) ≈ 2.6e10 FLOPs
(sparse: 4x less than the reference's dense form), vs 1.03e11 dense.
"""

import math
from contextlib import ExitStack

import numpy as np

import concourse.bass as bass
import concourse.tile as tile
from concourse import bacc, mybir
from concourse.bass_utils import run_bass_kernel_spmd

D = 1024       # model dim
H = 1024       # per-expert hidden dim
E = 8          # experts == cores
TG = 512       # tokens per tile chunk (full PSUM bank; float32r full rate >= 256)
MM_DT = mybir.dt.float32r
ACC_DT = mybir.dt.float32

_BUILD_CACHE: dict[int, object] = {}


def _chunks(C):
    """Split C tokens into chunks of 512 plus one ragged 128-multiple tail."""
    out = [TG] * (C // TG)
    if C % TG:
        out.append(C % TG)
    return out


def _build(C):
    """Build + compile the single-expert SwiGLU FFN program for capacity C.

    Computes yt = (silu(xt.T @ pw) * (xt.T @ gw)) @ ow, transposed:
    everything is laid out [feature, token] so no on-device transposes are
    needed (host ships x pre-transposed and un-transposes y).
    """
    assert C % 128 == 0
    nc = bacc.Bacc("TRN2", target_bir_lowering=False, debug=False, num_devices=E)
    xt = nc.dram_tensor("xt", [D, C], MM_DT, kind="ExternalInput").ap()
    gw = nc.dram_tensor("gw", [D, H], MM_DT, kind="ExternalInput").ap()
    pw = nc.dram_tensor("pw", [D, H], MM_DT, kind="ExternalInput").ap()
    ow = nc.dram_tensor("ow", [H, D], MM_DT, kind="ExternalInput").ap()
    yt = nc.dram_tensor("yt", [D, C], ACC_DT, kind="ExternalOutput").ap()

    KB = D // 128  # contraction blocks for the first matmuls
    HB = H // 128  # hidden blocks

    # Partition-blocked 3D views: [(a p) m] -> [p, a, m] so each weight
    # matrix / token chunk moves as ONE large DMA (HWDGE trigger is ~600ns
    # of engine time each; batching to >=1MiB is the documented rule).
    xt_r = xt.rearrange("(a p) c -> p a c", p=128)
    yt_r = yt.rearrange("(a p) c -> p a c", p=128)
    gw_r = gw.rearrange("(a p) m -> p a m", p=128)
    pw_r = pw.rearrange("(a p) m -> p a m", p=128)
    ow_r = ow.rearrange("(a p) m -> p a m", p=128)

    with tile.TileContext(nc) as tc, ExitStack() as ctx:
        wpool = ctx.enter_context(tc.tile_pool(name="w", bufs=1))
        xpool = ctx.enter_context(tc.tile_pool(name="x", bufs=2))
        hpool = ctx.enter_context(tc.tile_pool(name="h", bufs=2))
        spool = ctx.enter_context(tc.tile_pool(name="s", bufs=2))
        ypool = ctx.enter_context(tc.tile_pool(name="y", bufs=2))
        gpsum = ctx.enter_context(tc.tile_pool(name="pg", bufs=3, space="PSUM"))
        ppsum = ctx.enter_context(tc.tile_pool(name="pp", bufs=3, space="PSUM"))
        ypsum = ctx.enter_context(tc.tile_pool(name="py", bufs=2, space="PSUM"))

        def load_x(col, tg):
            t = xpool.tile([128, KB, tg], MM_DT, tag="x")
            nc.sync.dma_start(t[:], xt_r[:, :, col:col + tg])
            return t

        def load_w(ap_r, prefix):
            # two halves: G can start after the first 4 k-blocks land, and
            # the trigger pipeline (one HWDGE ring) interleaves better.
            t = wpool.tile([128, KB, H], MM_DT, tag=prefix)
            half = KB // 2
            nc.sync.dma_start(t[:, :half, :], ap_r[:, :half, :])
            nc.sync.dma_start(t[:, half:, :], ap_r[:, half:, :])
            return t

        # Issue order matters: PE starts on chunk 0's G matmuls as soon as
        # x(chunk0) + gw arrive; pw is needed ~14us later, ow ~28us later
        # (and the Y phase runs one chunk behind to hide ow's wire time).
        chunk_list = _chunks(C)
        xs_next = load_x(0, chunk_list[0])
        gw_t = load_w(gw_r, "gw")
        pw_t = load_w(pw_r, "pw")
        ow_t = load_w(ow_r, "ow")

        pending_y = None  # (hs, col, tg) of the previous chunk

        def emit_y(hs, ycol, tg):
            ybig = ypool.tile([128, KB, tg], ACC_DT, tag="y")
            for dblk in range(KB):
                py = ypsum.tile([128, tg], ACC_DT, tag="py")
                for h in range(HB):
                    nc.tensor.matmul(
                        py[:], ow_t[:, h, dblk * 128:(dblk + 1) * 128], hs[h][:],
                        start=(h == 0), stop=(h == HB - 1))
                nc.vector.tensor_copy(ybig[:, dblk, :], py[:])
            # ACT's HWDGE ring: keep SyncE's ring free for input feeding
            nc.scalar.dma_start(yt_r[:, :, ycol:ycol + tg], ybig[:])

        col = 0
        for ci, tg in enumerate(chunk_list):
            xs = xs_next
            if ci + 1 < len(chunk_list):
                xs_next = load_x(col + tg, chunk_list[ci + 1])

            hs = []
            for h in range(HB):
                pg = gpsum.tile([128, tg], ACC_DT, tag="pg")
                for k in range(KB):
                    nc.tensor.matmul(
                        pg[:], gw_t[:, k, h * 128:(h + 1) * 128], xs[:, k, :],
                        start=(k == 0), stop=(k == KB - 1))
                pp = ppsum.tile([128, tg], ACC_DT, tag="pp")
                for k in range(KB):
                    nc.tensor.matmul(
                        pp[:], pw_t[:, k, h * 128:(h + 1) * 128], xs[:, k, :],
                        start=(k == 0), stop=(k == KB - 1))
                sg = spool.tile([128, tg], ACC_DT, tag="sig")
                nc.scalar.activation(sg[:], pp[:], mybir.ActivationFunctionType.Sigmoid)
                sl = spool.tile([128, tg], ACC_DT, tag="sil")
                nc.vector.tensor_mul(sl[:], pp[:], sg[:])   # silu(p), one PSUM read
                ht = hpool.tile([128, tg], MM_DT, tag=f"h{h}")
                nc.vector.tensor_mul(ht[:], pg[:], sl[:])
                hs.append(ht)

            if pending_y is not None:
                emit_y(*pending_y)
            pending_y = (hs, col, tg)
            col += tg
        emit_y(*pending_y)

    nc.compile()
    return nc


def _get_program(C):
    if C not in _BUILD_CACHE:
        _BUILD_CACHE[C] = _build(C)
    return _BUILD_CACHE[C]


def _route(x, gate_w, k):
    """Router with the reference's exact jax ops (bitwise-matching top-k)."""
    import jax
    import jax.numpy as jnp

    router_logits = jnp.asarray(x) @ jnp.asarray(gate_w)
    routing_probs = jax.nn.softmax(router_logits.astype(jnp.float32), axis=-1)
    top_w, top_i = jax.lax.top_k(routing_probs, k)
    top_w = top_w / jnp.sum(top_w, axis=-1, keepdims=True)
    top_w = top_w.astype(jnp.float32)

    n_exp = gate_w.shape[1]
    expert_mask = jax.nn.one_hot(top_i, n_exp, dtype=jnp.float32)
    tokens_per_expert = jnp.mean(expert_mask, axis=0)
    router_prob_per_expert = jnp.mean(routing_probs, axis=0)
    bl_loss = jnp.sum(tokens_per_expert * router_prob_per_expert[None, :]) * n_exp

    return (np.asarray(router_logits), np.asarray(top_i), np.asarray(top_w),
            np.asarray(bl_loss))


def _run_device(nc, in_maps, trace=False, **kw):
    return run_bass_kernel_spmd(nc, in_maps, core_ids=list(range(E)),
                                trace=trace, **kw)


def kernel(hidden_states, gate_w, gw, pw, ow, top_k, _trace=False, _res_out=None):
    hidden_states = np.asarray(hidden_states, dtype=np.float32)
    gate_w = np.asarray(gate_w, dtype=np.float32)
    gw = np.ascontiguousarray(np.asarray(gw, dtype=np.float32))
    pw = np.ascontiguousarray(np.asarray(pw, dtype=np.float32))
    ow = np.ascontiguousarray(np.asarray(ow, dtype=np.float32))
    k = int(top_k)

    B, S, _ = hidden_states.shape
    x = hidden_states.reshape(-1, D)
    T = x.shape[0]

    router_logits, top_i, top_w, bl_loss = _route(x, gate_w, k)

    # --- host dispatch (the "all-to-all") ---
    counts = np.bincount(top_i.ravel(), minlength=E)
    C = max(128, int(math.ceil(counts.max() / 128)) * 128)
    prog = _get_program(C)

    xT = np.ascontiguousarray(x.T)  # [D, T] so per-expert gathers are column slices
    idx_list, w_list, in_maps = [], [], []
    for e in range(E):
        sel = top_i == e                       # [T, k]
        idx = np.nonzero(sel.any(axis=1))[0]   # tokens routed to e
        wgt = (top_w[idx] * sel[idx]).sum(axis=1).astype(np.float32)
        xt = np.zeros((D, C), np.float32)
        xt[:, :idx.size] = xT[:, idx]
        idx_list.append(idx)
        w_list.append(wgt)
        in_maps.append({"xt": xt, "gw": gw[e], "pw": pw[e], "ow": ow[e]})

    res = _run_device(prog, in_maps, trace=_trace)
    if _res_out is not None:
        _res_out.append(res)

    final = np.zeros((T, D), np.float32)
    for e in range(E):
        idx = idx_list[e]
        ye = res.results[e]["yt"][:, :idx.size].T  # [n_e, D]
        final[idx] += ye * w_list[e][:, None]

    return (final.reshape(B, S, D),
            router_logits,
            np.float32(bl_loss))
